# revision 11
# baseline (speedup 1.0000x reference)
"""Multi-head attention (B=2, S=2048, D=1024, H=16) on 8 trn2 NeuronCores.

Sharding: core c -> batch b = c // 4, head group g = c % 4 (heads 4g..4g+3).
Each core computes, for its batch shard and 4 heads:
  QT/KT = (x W + b)^T in [d_local, seq] layout, V in [seq, d_local] layout,
  transposed scores S^T[k, q] = K Q^T (so softmax needs no transposes),
  exp via ACT (scale fused), PV matmul with an appended ones column which
  yields both the unnormalized context and the softmax row sums,
  normalization via reciprocal + gpsimd partition-broadcast multiply,
  and a partial output projection against a row shard of Wo.
Host sums the 4 partials per batch and adds the constant row bv @ Wo + bo
(softmax rows sum to one, so bv's contribution is a constant vector).

Everything is bf16: fp8 projections cost ~1.9e-2 rel err (random-sign
contractions don't average quantization noise) and fp8 PV buys no time
because the scalar-engine exp stream (~142us) walls the attention phase
anyway.

Schedule notes:
 - Weights/x are host-prechunked to [128, ...] so every load is one
   contiguous DMA, ordered by consumption (wq, x, wk early; wv, wo late).
 - At each head boundary the first two score chunks of the NEXT head are
   emitted before the last PV pair + normalize of the current head, so the
   PE never idles waiting for the trailing exp stream (a PE stall also
   drops the clock to a lower p-state for ~3us, amplifying it).
 - The tail splits psum->sbuf output copies between ACT (idle after the
   last exp; Copy shares the exp activation table, no reload) and DVE.
"""

import sys

sys.path.insert(0, "/opt/trn_rl_repo")

import numpy as np
import ml_dtypes

B = 2
S = 2048
D = 1024
H = 16
HD = 64
NCORES = 8
HPC = 4          # heads per core
DL = HPC * HD    # 256 local head dims per core
P = 128
KCH = S // P     # 16 key chunks
DCH = D // P     # 8 contraction chunks
TBLK = S // P    # 16 token blocks
SCALE = 1.0 / np.sqrt(HD)

_CACHE = {}


def _build():
    import concourse.bass as bass  # noqa: F401
    import concourse.mybir as mybir
    import concourse.tile as tile
    from concourse import bacc

    bf16 = mybir.dt.bfloat16
    f32 = mybir.dt.float32
    Exp = mybir.ActivationFunctionType.Exp

    nc = bacc.Bacc("TRN2", target_bir_lowering=False, debug=False,
                   num_devices=NCORES)

    # host pre-chunks everything to [128, ...] so each load is one DMA of
    # contiguous per-partition lines
    xt_d = nc.dram_tensor("xt", [P, DCH * S], bf16, kind="ExternalInput")
    wq_d = nc.dram_tensor("wq", [P, DCH * DL], bf16, kind="ExternalInput")
    wk_d = nc.dram_tensor("wk", [P, DCH * DL], bf16, kind="ExternalInput")
    wv_d = nc.dram_tensor("wv", [P, DCH * DL], bf16, kind="ExternalInput")
    wo_d = nc.dram_tensor("wo", [P, 2 * D], bf16, kind="ExternalInput")
    bqk_d = nc.dram_tensor("bqk", [P, 4], f32, kind="ExternalInput")
    out_d = nc.dram_tensor("out", [S, D], bf16, kind="ExternalOutput")

    with tile.TileContext(nc) as tc:
        with (
            tc.tile_pool(name="persist", bufs=1) as pp,
            tc.tile_pool(name="stream", bufs=3) as sp,
            tc.tile_pool(name="psum", bufs=2, space="PSUM") as ps,
        ):
            # ---- input loads (consumption order; x chunks gate the front)
            bqk_s = pp.tile([P, 4], f32, tag="bqk", name="bqk_s")
            nc.sync.dma_start(bqk_s[:], bqk_d[:])
            wq_s = pp.tile([P, DCH * DL], bf16, tag="wq", name="wq_s")
            wk_s = pp.tile([P, DCH * DL], bf16, tag="wk", name="wk_s")
            wv_s = pp.tile([P, DCH * DL], bf16, tag="wv", name="wv_s")
            wo_s = pp.tile([P, 2 * D], bf16, tag="wo", name="wo_s")
            xt_s = pp.tile([P, DCH * S], bf16, tag="xt", name="xt_s")
            # sync queue: bqk, wq chunk 0, x0, wq rest, x2, x4, x6
            # gpsimd queue: wk, x1, x3, x5, x7, wv, wo
            # (wq chunk 0 + x0 + wk land first so the Q/K projection
            # interleave starts ~3us in; wv/wo aren't needed until later)
            nc.sync.dma_start(wq_s[:, 0:DL], wq_d[:, 0:DL])
            nc.gpsimd.dma_start(wk_s[:], wk_d[:])
            nc.sync.dma_start(xt_s[:, 0:S], xt_d[:, 0:S])
            nc.sync.dma_start(wq_s[:, DL:], wq_d[:, DL:])
            for c in (1, 2, 3, 4, 5, 6, 7):
                eng = nc.gpsimd if c % 2 == 1 else nc.sync
                eng.dma_start(xt_s[:, c * S:(c + 1) * S],
                              xt_d[:, c * S:(c + 1) * S])
            nc.gpsimd.dma_start(wv_s[:], wv_d[:])
            nc.gpsimd.dma_start(wo_s[:], wo_d[:])

            xt3 = xt_s.rearrange("p (c s) -> p c s", s=S)
            wq3 = wq_s.rearrange("p (c n) -> p c n", n=DL)
            wk3 = wk_s.rearrange("p (c n) -> p c n", n=DL)
            wv3 = wv_s.rearrange("p (c n) -> p c n", n=DL)
            wo3 = wo_s.rearrange("p (c n) -> p c n", n=D)

            qt = [None, None]
            kt = [None, None]

            def proj_alloc(which, dblk):
                nm = "qt" if which == 0 else "kt"
                t_sb = pp.tile([P, S], bf16, tag=f"{nm}{dblk}",
                               name=f"{nm}{dblk}")
                (qt if which == 0 else kt)[dblk] = t_sb

            def proj_half(which, dblk, half):
                """One projection half: 16 MMs + bias copy (short psum life)."""
                w3, bcol = (wq3, 0) if which == 0 else (wk3, 2)
                t_sb = (qt if which == 0 else kt)[dblk]
                acc = ps.tile([P, 1024], f32, tag="work",
                              name=f"ps_p{which}{dblk}{half}")
                for kc in range(DCH):
                    for ns in range(2):
                        nc.tensor.matmul(
                            acc[:, ns * 512:(ns + 1) * 512],
                            w3[:, kc, dblk * P:(dblk + 1) * P],
                            xt3[:, kc, half * 1024 + ns * 512:
                                half * 1024 + (ns + 1) * 512],
                            start=(kc == 0), stop=(kc == DCH - 1),
                        )
                nc.vector.tensor_scalar_add(
                    t_sb[:, half * 1024:(half + 1) * 1024],
                    acc[:],
                    bqk_s[:, bcol + dblk:bcol + dblk + 1],
                )

            def proj_front():
                """Q and K dblk0 interleaved per k-chunk so each x chunk is
                consumed once on arrival (the front is DMA-feed limited).
                K borrows the not-yet-used ctx psum tile as its accumulator
                (work tag only has 2 bufs)."""
                qaccs = [ps.tile([P, 1024], f32, tag="work",
                                 name=f"ps_fq{half}") for half in range(2)]
                kacc = ps.tile([P, S], f32, tag="ctx", bufs=1, name="ps_fk")
                for kc in range(DCH):
                    for half in range(2):
                        for ns in range(2):
                            nc.tensor.matmul(
                                qaccs[half][:, ns * 512:(ns + 1) * 512],
                                wq3[:, kc, 0:P],
                                xt3[:, kc, half * 1024 + ns * 512:
                                    half * 1024 + (ns + 1) * 512],
                                start=(kc == 0), stop=(kc == DCH - 1),
                            )
                        if kc == DCH - 1:
                            # 512-col bias adds, emitted right after each
                            # half's accumulation stops, so the first score
                            # chunk is unblocked ~1.5us after the last
                            # projection matmul instead of after a serial
                            # 5us DVE convoy
                            for ns in range(2):
                                col = half * 1024 + ns * 512
                                nc.vector.tensor_scalar_add(
                                    qt[0][:, col:col + 512],
                                    qaccs[half][:, ns * 512:(ns + 1) * 512],
                                    bqk_s[:, 0:1])
                    for half in range(2):
                        for ns in range(2):
                            col = half * 1024 + ns * 512
                            nc.tensor.matmul(
                                kacc[:, col:col + 512],
                                wk3[:, kc, 0:P],
                                xt3[:, kc, col:col + 512],
                                start=(kc == 0), stop=(kc == DCH - 1),
                            )
                        if kc == DCH - 1:
                            # K's adds ride the idle scalar engine (Identity
                            # shares the exp activation table - no reload)
                            for ns in range(2):
                                col = half * 1024 + ns * 512
                                nc.scalar.add(
                                    kt[0][:, col:col + 512],
                                    kacc[:, col:col + 512],
                                    bqk_s[:, 2:3])

            # V is stored in k-chunk PAIRS: [128, 2, 4 heads * 68] where col
            # 68h+64 holds the softmax-sum ones column (65..67 pad keeps the
            # pair step 16B-aligned).
            vts = [None] * (TBLK // 2)

            def v_proj(tb):
                pr, j = tb // 2, tb % 2
                if j == 0:
                    vt = pp.tile([P, 2, HPC * 68], bf16, tag=f"v{pr}",
                                 name=f"v{pr}")
                    v4 = vt.rearrange("p j (h e) -> p j h e", e=68)
                    nc.gpsimd.memset(v4[:, :, :, 64:65], 1.0)
                    vts[pr] = vt
                vt = vts[pr]
                v4 = vt.rearrange("p j (h e) -> p j h e", e=68)
                acc = ps.tile([P, 1024], f32, tag="work", name=f"ps_v{tb}")
                for kc in range(DCH):
                    nc.tensor.matmul(
                        acc[:, 0:DL],
                        xt3[:, kc, tb * P:(tb + 1) * P],
                        wv3[:, kc, :],
                        start=(kc == 0), stop=(kc == DCH - 1),
                    )
                nc.vector.tensor_copy(
                    v4[:, j, :, 0:64],
                    acc[:, 0:DL].rearrange("p (h e) -> p h e", e=64),
                )

            ctx_sb = [pp.tile([P, S], bf16, tag=f"ctx{dc}", name=f"ctx{dc}")
                      for dc in range(2)]
            ctx_ps_ref = [None]
            etps = [None] * (KCH // 2)

            def scores_chunk(h, kc):
                dblk = h // 2
                roff = 64 * (h % 2)
                pr, j = kc // 2, kc % 2
                if j == 0:
                    etps[pr] = sp.tile([P, 2, S], bf16, tag="expt", bufs=4,
                                       name=f"expt{h}_{pr}")
                et = etps[pr]
                for half in range(2):
                    sc = ps.tile([P, 1024], f32, tag="work",
                                 name=f"ps_sc{h}_{kc}_{half}")
                    for ns in range(2):
                        nc.tensor.matmul(
                            sc[:, ns * 512:(ns + 1) * 512],
                            kt[dblk][roff:roff + 64, kc * P:(kc + 1) * P],
                            qt[dblk][roff:roff + 64,
                                     half * 1024 + ns * 512:
                                     half * 1024 + (ns + 1) * 512],
                            start=True, stop=True,
                        )
                    nc.scalar.activation(
                        et[:, j, half * 1024:(half + 1) * 1024], sc[:],
                        Exp, scale=float(SCALE),
                    )

            NPAIR = KCH // 2

            def pv_pair(h, pr, js=(0, 1)):
                if pr == 0:
                    ctx_ps_ref[0] = ps.tile([P, S], f32, tag="ctx", bufs=1,
                                            name=f"ps_ctx{h}")
                ctx_ps = ctx_ps_ref[0]
                v4 = vts[pr].rearrange("p j (h e) -> p j h e", e=68)
                for j in js:
                    for ns in range(4):
                        nc.tensor.matmul(
                            ctx_ps[0:65, ns * 512:(ns + 1) * 512],
                            v4[:, j, h, 0:65],
                            etps[pr][:, j, ns * 512:(ns + 1) * 512],
                            start=(pr == 0 and j == 0),
                            stop=(pr == NPAIR - 1 and j == 1),
                        )

            def normalize(h, part=0, nparts=2):
                """Normalize one 1/nparts slice of head h's context."""
                dblk = h // 2
                roff = 64 * (h % 2)
                w = S // nparts
                ctx_ps = ctx_ps_ref[0]
                hs = slice(part * w, (part + 1) * w)
                srow = sp.tile([1, w], f32, tag=f"srow{w}", bufs=2,
                               name=f"srow{h}_{part}")
                nc.vector.tensor_copy(srow[:], ctx_ps[64:65, hs])
                rec = sp.tile([1, w], f32, tag=f"rec{w}", bufs=2,
                              name=f"rec{h}_{part}")
                nc.vector.reciprocal_approx_fast(rec[:], srow[:])
                bc = sp.tile([64, w], f32, tag=f"bc{w}", bufs=2,
                             name=f"bc{h}_{part}")
                nc.gpsimd.partition_broadcast(bc[:], rec[:])
                nc.vector.tensor_mul(
                    ctx_sb[dblk][roff:roff + 64, hs],
                    ctx_ps[0:64, hs], bc[:])

            # ---- emission schedule ----
            # dense front: Q+K dblk0 interleaved, streaming against the x DMA
            proj_alloc(0, 0)
            proj_alloc(1, 0)
            proj_front()
            proj_alloc(0, 1)
            proj_alloc(1, 1)
            # fillers: h0 -> V proj per chunk shifted +2 (wv lands late);
            # h1 -> the four dblk1 projection halves (h1 reads dblk0)
            h1_fill = [(0, 1, 0), (0, 1, 1), (1, 1, 0), (1, 1, 1)]

            def head_chunk(h, kc):
                if h == 0 and kc >= 2:
                    v_proj(kc - 2)
                elif h == 1 and kc % 4 == 0:
                    proj_half(*h1_fill[kc // 4])
                scores_chunk(h, kc)

            # PV emission plan (chunk-pair indices): the PV stream starts 5
            # scores-chunks in, so the previous head's normalize chain always
            # has PE runway.  Pairs 6 and 7 are emitted in the transition
            # block: pair 6 + the next head's first score chunk give the PE
            # ~2.5us of runway that hides the trailing exp(15) latency, so
            # pair 7 never stalls (a stall also drops the PE clock p-state).
            pv_plan = {5: [0], 6: [1], 7: [2], 9: [3], 11: [4], 13: [5]}
            for h in range(HPC):
                for kc in range(2 if h > 0 else 0, KCH):
                    head_chunk(h, kc)
                    for pkc in pv_plan.get(kc, []):
                        pv_pair(h, pkc)
                if h == 0:
                    v_proj(14)
                    v_proj(15)
                pv_pair(h, NPAIR - 2)
                if h < HPC - 1:
                    head_chunk(h + 1, 0)
                    pv_pair(h, NPAIR - 1)
                    head_chunk(h + 1, 1)
                    normalize(h, 0, 2)
                    normalize(h, 1, 2)
                else:
                    # no next-head scores to hide the trailing exp: split the
                    # last pair by k-tile so its first half (needs exp 14
                    # only) runs while exp 15 finishes
                    pv_pair(h, NPAIR - 1, js=(0,))
                    pv_pair(h, NPAIR - 1, js=(1,))

            # ---- last-head normalize (quartered) + output projection ----
            def out_tb(tb):
                acc = ps.tile([P, 1024], f32, tag="work", name=f"ps_o{tb}")
                for dc in range(2):
                    for ns in range(2):
                        nc.tensor.matmul(
                            acc[:, ns * 512:(ns + 1) * 512],
                            ctx_sb[dc][:, tb * P:(tb + 1) * P],
                            wo3[:, dc, ns * 512:(ns + 1) * 512],
                            start=(dc == 0), stop=(dc == 1),
                        )
                o_sb = sp.tile([P, D], bf16, tag="osb", name=f"osb{tb}")
                # the psum->sbuf copies ride mostly on ACT (idle after the
                # last exp); DVE takes every 4th plus the normalize chains
                if tb % 4 == 3:
                    nc.vector.tensor_copy(o_sb[:], acc[:])
                else:
                    nc.scalar.copy(o_sb[:], acc[:])
                eng = nc.sync if tb % 2 == 0 else nc.gpsimd
                eng.dma_start(out_d[tb * P:(tb + 1) * P, :], o_sb[:])

            # normalize chains run one quartet ahead of the out stream
            normalize(HPC - 1, 0, 4)
            normalize(HPC - 1, 1, 4)
            for tb in range(0, 4):
                out_tb(tb)
            normalize(HPC - 1, 2, 4)
            for tb in range(4, 8):
                out_tb(tb)
            normalize(HPC - 1, 3, 4)
            for tb in range(8, 16):
                out_tb(tb)

    nc.compile()
    return nc


def _get_compiled():
    if "nc" not in _CACHE:
        _CACHE["nc"] = _build()
    return _CACHE["nc"]


def kernel(x, Wq, bq, Wk, bk, Wv, bv, Wo, bo):
    from concourse.bass_utils import run_bass_kernel_spmd

    nc = _get_compiled()
    x = np.asarray(x, dtype=np.float32)
    Wq, bq = np.asarray(Wq, np.float32), np.asarray(bq, np.float32)
    Wk, bk = np.asarray(Wk, np.float32), np.asarray(bk, np.float32)
    Wv, bv = np.asarray(Wv, np.float32), np.asarray(bv, np.float32)
    Wo, bo = np.asarray(Wo, np.float32), np.asarray(bo, np.float32)

    bf = ml_dtypes.bfloat16

    def chunk_rows(a):
        # [R, C] -> [128, (R/128)*C] grouping rows into 128-partitions
        r, c = a.shape
        return np.ascontiguousarray(
            a.reshape(r // P, P, c).transpose(1, 0, 2).reshape(P, -1))

    in_maps = []
    xts = [chunk_rows(np.ascontiguousarray(x[b].T)).astype(bf)
           for b in range(B)]
    for c in range(NCORES):
        b, g = c // 4, c % 4
        cols = slice(g * DL, (g + 1) * DL)
        bq_l, bk_l = bq[cols], bk[cols]
        bqk = np.stack(
            [bq_l[0:P], bq_l[P:2 * P], bk_l[0:P], bk_l[P:2 * P]], axis=1)
        in_maps.append({
            "xt": xts[b],
            "wq": chunk_rows(Wq[:, cols]).astype(bf),
            "wk": chunk_rows(Wk[:, cols]).astype(bf),
            "wv": chunk_rows(Wv[:, cols]).astype(bf),
            "wo": chunk_rows(Wo[cols, :]).astype(bf),
            "bqk": np.ascontiguousarray(bqk, np.float32),
        })

    _CACHE["in_maps"] = in_maps
    res = run_bass_kernel_spmd(nc, in_maps, list(range(NCORES)))

    # constant row: bv @ Wo + bo (softmax rows sum to 1)
    const_row = bv.astype(np.float64) @ Wo.astype(np.float64) + bo
    out = np.zeros((B, S, D), np.float64)
    for c in range(NCORES):
        out[c // 4] += res.results[c]["out"].astype(np.float64)
    out += const_row
    return out.astype(np.float32)


# revision 14
# speedup vs baseline: 1.0198x; 1.0198x over previous
"""Multi-head attention (B=2, S=2048, D=1024, H=16) on 8 trn2 NeuronCores.

Sharding: core c -> batch b = c // 4, head group g = c % 4 (heads 4g..4g+3).
Each core computes, for its batch shard and 4 heads:
  QT/KT = (x W + b)^T in [d_local, seq] layout, V in [seq, d_local] layout,
  transposed scores S^T[k, q] = K Q^T (so softmax needs no transposes),
  exp via ACT (scale fused), PV matmul with an appended ones column which
  yields both the unnormalized context and the softmax row sums,
  normalization via reciprocal + gpsimd partition-broadcast multiply,
  and a partial output projection against a row shard of Wo.
Host sums the 4 partials per batch and adds the constant row bv @ Wo + bo
(softmax rows sum to one, so bv's contribution is a constant vector).

Everything is bf16: fp8 projections cost ~1.9e-2 rel err (random-sign
contractions don't average quantization noise) and fp8 PV buys no time
because the scalar-engine exp stream (~142us) walls the attention phase
anyway.

Schedule notes:
 - Weights/x are host-prechunked to [128, ...] so every load is one
   contiguous DMA, ordered by consumption (wq, x, wk early; wv, wo late).
 - At each head boundary the first two score chunks of the NEXT head are
   emitted before the last PV pair + normalize of the current head, so the
   PE never idles waiting for the trailing exp stream (a PE stall also
   drops the clock to a lower p-state for ~3us, amplifying it).
 - The tail splits psum->sbuf output copies between ACT (idle after the
   last exp; Copy shares the exp activation table, no reload) and DVE.
"""

import sys

sys.path.insert(0, "/opt/trn_rl_repo")

import numpy as np
import ml_dtypes

B = 2
S = 2048
D = 1024
H = 16
HD = 64
NCORES = 8
HPC = 4          # heads per core
DL = HPC * HD    # 256 local head dims per core
P = 128
KCH = S // P     # 16 key chunks
DCH = D // P     # 8 contraction chunks
TBLK = S // P    # 16 token blocks
SCALE = 1.0 / np.sqrt(HD)

_CACHE = {}


def _build():
    import concourse.bass as bass  # noqa: F401
    import concourse.mybir as mybir
    import concourse.tile as tile
    from concourse import bacc

    bf16 = mybir.dt.bfloat16
    f32 = mybir.dt.float32
    Exp = mybir.ActivationFunctionType.Exp

    nc = bacc.Bacc("TRN2", target_bir_lowering=False, debug=False,
                   num_devices=NCORES)

    # host pre-chunks everything to [128, ...] so each load is one DMA of
    # contiguous per-partition lines
    xt_d = nc.dram_tensor("xt", [P, DCH * S], bf16, kind="ExternalInput")
    wq_d = nc.dram_tensor("wq", [P, DCH * DL], bf16, kind="ExternalInput")
    wk_d = nc.dram_tensor("wk", [P, DCH * DL], bf16, kind="ExternalInput")
    wv_d = nc.dram_tensor("wv", [P, DCH * DL], bf16, kind="ExternalInput")
    wo_d = nc.dram_tensor("wo", [P, 2 * D], bf16, kind="ExternalInput")
    bqk_d = nc.dram_tensor("bqk", [P, 4], f32, kind="ExternalInput")
    out_d = nc.dram_tensor("out", [S, D], bf16, kind="ExternalOutput")

    with tile.TileContext(nc) as tc:
        with (
            tc.tile_pool(name="persist", bufs=1) as pp,
            tc.tile_pool(name="stream", bufs=3) as sp,
            tc.tile_pool(name="psum", bufs=2, space="PSUM") as ps,
        ):
            # ---- input loads (consumption order; x chunks gate the front)
            bqk_s = pp.tile([P, 4], f32, tag="bqk", name="bqk_s")
            nc.sync.dma_start(bqk_s[:], bqk_d[:])
            wq_s = pp.tile([P, DCH * DL], bf16, tag="wq", name="wq_s")
            wk_s = pp.tile([P, DCH * DL], bf16, tag="wk", name="wk_s")
            wv_s = pp.tile([P, DCH * DL], bf16, tag="wv", name="wv_s")
            wo_s = pp.tile([P, 2 * D], bf16, tag="wo", name="wo_s")
            xt_s = pp.tile([P, DCH * S], bf16, tag="xt", name="xt_s")
            # sync queue: bqk, wq chunk 0, x0, wq rest, x2, x4, x6
            # gpsimd queue: wk, x1, x3, x5, x7, wv, wo
            # (wq chunk 0 + x0 + wk land first so the Q/K projection
            # interleave starts ~3us in; wv/wo aren't needed until later)
            nc.sync.dma_start(wq_s[:, 0:DL], wq_d[:, 0:DL])
            nc.gpsimd.dma_start(wk_s[:], wk_d[:])
            nc.sync.dma_start(xt_s[:, 0:S], xt_d[:, 0:S])
            nc.sync.dma_start(wq_s[:, DL:], wq_d[:, DL:])
            for c in (1, 2, 3, 4, 5, 6, 7):
                eng = nc.gpsimd if c % 2 == 1 else nc.sync
                eng.dma_start(xt_s[:, c * S:(c + 1) * S],
                              xt_d[:, c * S:(c + 1) * S])
            nc.gpsimd.dma_start(wv_s[:], wv_d[:])
            nc.gpsimd.dma_start(wo_s[:], wo_d[:])

            xt3 = xt_s.rearrange("p (c s) -> p c s", s=S)
            wq3 = wq_s.rearrange("p (c n) -> p c n", n=DL)
            wk3 = wk_s.rearrange("p (c n) -> p c n", n=DL)
            wv3 = wv_s.rearrange("p (c n) -> p c n", n=DL)
            wo3 = wo_s.rearrange("p (c n) -> p c n", n=D)

            qt = [None, None]
            kt = [None, None]

            def proj_alloc(which, dblk):
                nm = "qt" if which == 0 else "kt"
                t_sb = pp.tile([P, S], bf16, tag=f"{nm}{dblk}",
                               name=f"{nm}{dblk}")
                (qt if which == 0 else kt)[dblk] = t_sb

            def proj_half(which, dblk, half):
                """One projection half: 16 MMs + bias copy (short psum life)."""
                w3, bcol = (wq3, 0) if which == 0 else (wk3, 2)
                t_sb = (qt if which == 0 else kt)[dblk]
                acc = ps.tile([P, 1024], f32, tag="work",
                              name=f"ps_p{which}{dblk}{half}")
                for kc in range(DCH):
                    for ns in range(2):
                        nc.tensor.matmul(
                            acc[:, ns * 512:(ns + 1) * 512],
                            w3[:, kc, dblk * P:(dblk + 1) * P],
                            xt3[:, kc, half * 1024 + ns * 512:
                                half * 1024 + (ns + 1) * 512],
                            start=(kc == 0), stop=(kc == DCH - 1),
                        )
                nc.vector.tensor_scalar_add(
                    t_sb[:, half * 1024:(half + 1) * 1024],
                    acc[:],
                    bqk_s[:, bcol + dblk:bcol + dblk + 1],
                )

            def proj_front():
                """Q and K dblk0 interleaved per k-chunk so each x chunk is
                consumed once on arrival (the front is DMA-feed limited).
                K borrows the not-yet-used ctx psum tile as its accumulator
                (work tag only has 2 bufs)."""
                qaccs = [ps.tile([P, 1024], f32, tag="work",
                                 name=f"ps_fq{half}") for half in range(2)]
                kacc = ps.tile([P, S], f32, tag="ctx", bufs=1, name="ps_fk")
                for kc in range(DCH):
                    for half in range(2):
                        for ns in range(2):
                            nc.tensor.matmul(
                                qaccs[half][:, ns * 512:(ns + 1) * 512],
                                wq3[:, kc, 0:P],
                                xt3[:, kc, half * 1024 + ns * 512:
                                    half * 1024 + (ns + 1) * 512],
                                start=(kc == 0), stop=(kc == DCH - 1),
                            )
                        if kc == DCH - 1:
                            # 512-col bias adds, emitted right after each
                            # half's accumulation stops, so the first score
                            # chunk is unblocked ~1.5us after the last
                            # projection matmul instead of after a serial
                            # 5us DVE convoy
                            for ns in range(2):
                                col = half * 1024 + ns * 512
                                nc.vector.tensor_scalar_add(
                                    qt[0][:, col:col + 512],
                                    qaccs[half][:, ns * 512:(ns + 1) * 512],
                                    bqk_s[:, 0:1])
                    for half in range(2):
                        for ns in range(2):
                            col = half * 1024 + ns * 512
                            nc.tensor.matmul(
                                kacc[:, col:col + 512],
                                wk3[:, kc, 0:P],
                                xt3[:, kc, col:col + 512],
                                start=(kc == 0), stop=(kc == DCH - 1),
                            )
                        if kc == DCH - 1:
                            # K's adds ride the idle scalar engine (Identity
                            # shares the exp activation table - no reload)
                            for ns in range(2):
                                col = half * 1024 + ns * 512
                                nc.scalar.add(
                                    kt[0][:, col:col + 512],
                                    kacc[:, col:col + 512],
                                    bqk_s[:, 2:3])

            # V is stored in k-chunk PAIRS: [128, 2, 4 heads * 68] where col
            # 68h+64 holds the softmax-sum ones column (65..67 pad keeps the
            # pair step 16B-aligned).
            vts = [None] * (TBLK // 2)

            def v_proj(tb):
                pr, j = tb // 2, tb % 2
                if j == 0:
                    vt = pp.tile([P, 2, HPC * 68], bf16, tag=f"v{pr}",
                                 name=f"v{pr}")
                    v4 = vt.rearrange("p j (h e) -> p j h e", e=68)
                    nc.gpsimd.memset(v4[:, :, :, 64:65], 1.0)
                    vts[pr] = vt
                vt = vts[pr]
                v4 = vt.rearrange("p j (h e) -> p j h e", e=68)
                acc = ps.tile([P, 1024], f32, tag="work", name=f"ps_v{tb}")
                for kc in range(DCH):
                    nc.tensor.matmul(
                        acc[:, 0:DL],
                        xt3[:, kc, tb * P:(tb + 1) * P],
                        wv3[:, kc, :],
                        start=(kc == 0), stop=(kc == DCH - 1),
                    )
                nc.vector.tensor_copy(
                    v4[:, j, :, 0:64],
                    acc[:, 0:DL].rearrange("p (h e) -> p h e", e=64),
                )

            ctx_sb = [pp.tile([P, S], bf16, tag=f"ctx{dc}", name=f"ctx{dc}")
                      for dc in range(2)]
            ctx_ps_ref = [None]
            etps = [None] * (KCH // 2)

            def scores_chunk(h, kc):
                dblk = h // 2
                roff = 64 * (h % 2)
                pr, j = kc // 2, kc % 2
                if j == 0:
                    etps[pr] = sp.tile([P, 2, S], bf16, tag="expt", bufs=4,
                                       name=f"expt{h}_{pr}")
                et = etps[pr]
                for half in range(2):
                    sc = ps.tile([P, 1024], f32, tag="work",
                                 name=f"ps_sc{h}_{kc}_{half}")
                    for ns in range(2):
                        nc.tensor.matmul(
                            sc[:, ns * 512:(ns + 1) * 512],
                            kt[dblk][roff:roff + 64, kc * P:(kc + 1) * P],
                            qt[dblk][roff:roff + 64,
                                     half * 1024 + ns * 512:
                                     half * 1024 + (ns + 1) * 512],
                            start=True, stop=True,
                        )
                    nc.scalar.activation(
                        et[:, j, half * 1024:(half + 1) * 1024], sc[:],
                        Exp, scale=float(SCALE),
                    )

            NPAIR = KCH // 2

            def pv_pair(h, pr, js=(0, 1)):
                if pr == 0:
                    ctx_ps_ref[0] = ps.tile([P, S], f32, tag="ctx", bufs=1,
                                            name=f"ps_ctx{h}")
                ctx_ps = ctx_ps_ref[0]
                v4 = vts[pr].rearrange("p j (h e) -> p j h e", e=68)
                for j in js:
                    for ns in range(4):
                        nc.tensor.matmul(
                            ctx_ps[0:65, ns * 512:(ns + 1) * 512],
                            v4[:, j, h, 0:65],
                            etps[pr][:, j, ns * 512:(ns + 1) * 512],
                            start=(pr == 0 and j == 0),
                            stop=(pr == NPAIR - 1 and j == 1),
                        )

            def normalize(h, part=0, nparts=2):
                """Normalize one 1/nparts slice of head h's context."""
                dblk = h // 2
                roff = 64 * (h % 2)
                w = S // nparts
                ctx_ps = ctx_ps_ref[0]
                hs = slice(part * w, (part + 1) * w)
                srow = sp.tile([1, w], f32, tag=f"srow{w}", bufs=2,
                               name=f"srow{h}_{part}")
                nc.vector.tensor_copy(srow[:], ctx_ps[64:65, hs])
                rec = sp.tile([1, w], f32, tag=f"rec{w}", bufs=2,
                              name=f"rec{h}_{part}")
                nc.vector.reciprocal_approx_fast(rec[:], srow[:])
                bc = sp.tile([64, w], f32, tag=f"bc{w}", bufs=2,
                             name=f"bc{h}_{part}")
                nc.gpsimd.partition_broadcast(bc[:], rec[:])
                nc.vector.tensor_mul(
                    ctx_sb[dblk][roff:roff + 64, hs],
                    ctx_ps[0:64, hs], bc[:])

            # ---- emission schedule ----
            # dense front: Q+K dblk0 interleaved, streaming against the x DMA
            proj_alloc(0, 0)
            proj_alloc(1, 0)
            proj_front()
            proj_alloc(0, 1)
            proj_alloc(1, 1)
            # fillers: h0 -> V proj per chunk shifted +2 (wv lands late);
            # h1 -> the four dblk1 projection halves (h1 reads dblk0)
            h1_fill = [(0, 1, 0), (0, 1, 1), (1, 1, 0), (1, 1, 1)]

            def head_chunk(h, kc):
                if h == 0:
                    v_proj(kc)
                elif h == 1 and kc % 4 == 0:
                    proj_half(*h1_fill[kc // 4])
                scores_chunk(h, kc)

            # PV emission plan (chunk-pair indices): the PV stream starts 5
            # scores-chunks in, so the previous head's normalize chain always
            # has PE runway.  Pairs 4-7 are emitted in the transition block:
            # by a head's end the exp stream trails the scores by up to two
            # chunks (~5us, the work-psum leash), so pairs 4-6 plus the next
            # head's first score chunk give the PE ~6.8us of exp-independent
            # runway before pair 7 needs exp(15).  That same runway lets the
            # ACT stream advance into the next head's first exps, so the
            # next score chunks don't stall either (a PE stall also drops
            # the clock p-state for ~10us).
            pv_plan = {5: [0], 6: [1], 7: [2], 9: [3]}
            for h in range(HPC):
                for kc in range(2 if h > 0 else 0, KCH):
                    head_chunk(h, kc)
                    for pkc in pv_plan.get(kc, []):
                        pv_pair(h, pkc)
                pv_pair(h, 4)
                pv_pair(h, 5)
                pv_pair(h, 6)
                if h < HPC - 1:
                    head_chunk(h + 1, 0)
                    pv_pair(h, 7, js=(0,))
                    pv_pair(h, 7, js=(1,))
                    normalize(h, 0, 2)
                    normalize(h, 1, 2)
                    head_chunk(h + 1, 1)
                else:
                    pv_pair(h, 7, js=(0,))
                    pv_pair(h, 7, js=(1,))

            # ---- last-head normalize (quartered) + output projection ----
            def out_tb(tb, acc=None):
                if acc is None:
                    acc = ps.tile([P, 1024], f32, tag="work",
                                  name=f"ps_o{tb}")
                for dc in range(2):
                    for ns in range(2):
                        nc.tensor.matmul(
                            acc[:, ns * 512:(ns + 1) * 512],
                            ctx_sb[dc][:, tb * P:(tb + 1) * P],
                            wo3[:, dc, ns * 512:(ns + 1) * 512],
                            start=(dc == 0), stop=(dc == 1),
                        )
                o_sb = sp.tile([P, D], bf16, tag="osb", name=f"osb{tb}")
                # split the psum->sbuf copies between ACT (idle after the
                # last exp) and DVE so neither walls the tail
                if tb % 2 == 0:
                    nc.scalar.copy(o_sb[:], acc[:])
                else:
                    nc.vector.tensor_copy(o_sb[:], acc[:])
                eng = nc.sync if tb % 2 == 0 else nc.gpsimd
                eng.dma_start(out_d[tb * P:(tb + 1) * P, :], o_sb[:])

            # normalize chains run one quartet ahead of the out stream
            normalize(HPC - 1, 0, 4)
            normalize(HPC - 1, 1, 4)
            for tb in range(0, 4):
                out_tb(tb)
            normalize(HPC - 1, 2, 4)
            for tb in range(4, 8):
                out_tb(tb)
            normalize(HPC - 1, 3, 4)
            # ctx psum banks are free once the last normalize has read them:
            # borrow them as two extra accumulators so the final out blocks
            # aren't leashed to the copy stream by the 2-buf work tag
            ctx2 = ps.tile([P, S], f32, tag="ctx", bufs=1, name="ps_ctx_out")
            for tb in range(8, 16):
                if tb % 4 < 2:
                    out_tb(tb, ctx2[:, (tb % 2) * 1024:(tb % 2 + 1) * 1024])
                else:
                    out_tb(tb)

    nc.compile()
    return nc


def _get_compiled():
    if "nc" not in _CACHE:
        _CACHE["nc"] = _build()
    return _CACHE["nc"]


def kernel(x, Wq, bq, Wk, bk, Wv, bv, Wo, bo):
    from concourse.bass_utils import run_bass_kernel_spmd

    nc = _get_compiled()
    x = np.asarray(x, dtype=np.float32)
    Wq, bq = np.asarray(Wq, np.float32), np.asarray(bq, np.float32)
    Wk, bk = np.asarray(Wk, np.float32), np.asarray(bk, np.float32)
    Wv, bv = np.asarray(Wv, np.float32), np.asarray(bv, np.float32)
    Wo, bo = np.asarray(Wo, np.float32), np.asarray(bo, np.float32)

    bf = ml_dtypes.bfloat16

    def chunk_rows(a):
        # [R, C] -> [128, (R/128)*C] grouping rows into 128-partitions
        r, c = a.shape
        return np.ascontiguousarray(
            a.reshape(r // P, P, c).transpose(1, 0, 2).reshape(P, -1))

    in_maps = []
    xts = [chunk_rows(np.ascontiguousarray(x[b].T)).astype(bf)
           for b in range(B)]
    for c in range(NCORES):
        b, g = c // 4, c % 4
        cols = slice(g * DL, (g + 1) * DL)
        bq_l, bk_l = bq[cols], bk[cols]
        bqk = np.stack(
            [bq_l[0:P], bq_l[P:2 * P], bk_l[0:P], bk_l[P:2 * P]], axis=1)
        in_maps.append({
            "xt": xts[b],
            "wq": chunk_rows(Wq[:, cols]).astype(bf),
            "wk": chunk_rows(Wk[:, cols]).astype(bf),
            "wv": chunk_rows(Wv[:, cols]).astype(bf),
            "wo": chunk_rows(Wo[cols, :]).astype(bf),
            "bqk": np.ascontiguousarray(bqk, np.float32),
        })

    _CACHE["in_maps"] = in_maps
    res = run_bass_kernel_spmd(nc, in_maps, list(range(NCORES)))

    # constant row: bv @ Wo + bo (softmax rows sum to 1)
    const_row = bv.astype(np.float64) @ Wo.astype(np.float64) + bo
    out = np.zeros((B, S, D), np.float64)
    for c in range(NCORES):
        out[c // 4] += res.results[c]["out"].astype(np.float64)
    out += const_row
    return out.astype(np.float32)
